# revision 20
# baseline (speedup 1.0000x reference)
"""MobileMamba block kernel for 8x Trainium2 NeuronCores.

Math restructure of the reference:
  xc   = silu(x @ w1.T + b1)                          # [L, E]
  c    = depthwise_conv5(xc) (+bd, BN affine folded)  # [E, L]
  xl   = silu(c)                                      # BN folded into taps/bias
  SSM with constant B/C collapses to a scalar first-order recurrence:
    g[e,t] = expA[e]*g[e,t-1] + xl[e,t]
    ys[e,t] = CB[e]*g[e,t] + Dv[e]*xl[e,t],  CB = sum_s Bm*Cm
  out  = ys @ w2.T + b2   (CB/Dv folded into two pre-scaled w2.T copies)

Sharding: data-parallel over batch (B=8 -> 8 cores). Each core computes one
sample entirely in [channel, time] layout; the host pre-transposes x shards
and post-transposes outputs.

Device pipeline per core (one batch sample):
  mm1 on TensorE (fp32r), silu on ScalarE, depthwise conv as 5 PSUM-
  accumulated diagonal matmuls (bf16) on TensorE, the SSM recurrence as
  chunked tensor_tensor_scan on VectorE (carry chained through the previous
  chunk's last column), mm2 (two-path: g and xl) on TensorE in bf16.

All constants arrive via 3 single-DMA mega tensors; tiny per-engine "touch"
ops observe those DMA semaphores early so every real instruction carries at
most one sync wait (walrus allows a single wait command per instruction;
_split_waits spills any excess onto same-engine NoOps).
"""

import sys

for _p in ('/opt/trn_rl_repo',):
    if _p not in sys.path:
        sys.path.append(_p)

import numpy as np

import concourse.bass as bass
import concourse.tile as tile
from concourse import mybir

D = 256      # model dim
E = 512      # expanded dim
L = 2048     # sequence length
B = 8        # batch
NCORES = 8
BN_EPS = 1e-5

F32 = mybir.dt.float32
F32R = mybir.dt.float32r
BF16 = mybir.dt.bfloat16

EM = E // 128   # 4 channel tiles
DM = D // 128   # 2 model-dim tiles

# param-table columns (per channel): taps 0..4, conv/bn bias, b1, expA, CB/Dv
PT_CBIAS = 5
PT_B1 = 6
PT_EXPA = 7
PT_CBDV = 8
PT_NCOL = 9

MW_COLS = DM * 512                     # w1t chunks (fp32r)
MD_COLS = EM * 5 * 128 + EM * 256      # diag blocks + w2dv (bf16)
MP_COLS = EM * PT_NCOL + DM
W2DV0 = EM * 5 * 128


def _bcast(col_ap, n):
    """Broadcast a [128,1] per-partition column AP along the free dim."""
    return bass.AP(tensor=col_ap.tensor, offset=col_ap.offset,
                   ap=[col_ap.ap[0], [0, n]])


def build_nc(L=L, wsplit=True):
    nc = bass.Bass()
    xt = nc.declare_dram_parameter("xt", [D, L], F32, isOutput=False)
    mw = nc.declare_dram_parameter("mw", [128, MW_COLS], F32R, isOutput=False)
    md = nc.declare_dram_parameter("md", [128, MD_COLS], BF16, isOutput=False)
    mp = nc.declare_dram_parameter("mp", [128, MP_COLS], F32, isOutput=False)
    outT = nc.declare_dram_parameter("outT", [D, L], F32, isOutput=True)

    CH = min(512, L)
    LC = L // CH
    TAPS = (0, -1, 1, -2, 2)   # center first: start=True covers full range

    with tile.TileContext(nc) as tc:
        with (
            tc.tile_pool(name="const", bufs=1) as const,
            tc.tile_pool(name="acts", bufs=1) as acts,
            tc.tile_pool(name="psA", bufs=2, space="PSUM") as psA,
            tc.tile_pool(name="psB", bufs=4, space="PSUM") as psB,
            tc.tile_pool(name="psC", bufs=2, space="PSUM") as psC,
        ):
            # ---- constants: one DMA each ----
            mw_t = const.tile([128, MW_COLS], F32R)
            nc.sync.dma_start(out=mw_t, in_=mw[:, :])
            mp_t = const.tile([128, MP_COLS], F32)
            nc.sync.dma_start(out=mp_t, in_=mp[:, :])

            # ---- x load, chunked, lc-major so mm1 can start early ----
            xts = [acts.tile([128, L], F32R, name=f"xts{k}", tag=f"xt{k}")
                   for k in range(DM)]
            md_t = const.tile([128, MD_COLS], BF16)
            for lc in range(LC):
                for k in range(DM):
                    nc.sync.dma_start(
                        out=xts[k][:, lc * CH:(lc + 1) * CH],
                        in_=xt[k * 128:(k + 1) * 128,
                               lc * CH:(lc + 1) * CH].bitcast(F32R))
                if lc == 0:
                    nc.sync.dma_start(out=md_t, in_=md[:, :])

            # ---- per-engine touches (observe const DMA sems early) ----
            ps_scr = psA.tile([128, 8], F32, name="ps_scr", tag="ps1")
            nc.tensor.matmul(out=ps_scr[:, 0:4], lhsT=mw_t[:, 0:128],
                             rhs=mw_t[:, 0:4], start=True, stop=True)
            v_scr = const.tile([128, 1], F32)
            nc.vector.tensor_copy(out=v_scr, in_=mp_t[:, 0:1])
            a_scr = const.tile([128, 1], F32)
            nc.scalar.copy(out=a_scr, in_=mp_t[:, 0:1])

            # ---- constant slices ----
            w1s = [mw_t[:, k * 512:(k + 1) * 512] for k in range(DM)]
            diag = [[md_t[:, (m * 5 + j) * 128:(m * 5 + j + 1) * 128]
                     for j in range(5)] for m in range(EM)]
            w2dvs = [md_t[:, W2DV0 + ec * 256:W2DV0 + (ec + 1) * 256]
                     for ec in range(EM)]
            pts = [mp_t[:, m * PT_NCOL:(m + 1) * PT_NCOL] for m in range(EM)]
            b2s = [mp_t[:, EM * PT_NCOL + dt_:EM * PT_NCOL + dt_ + 1]
                   for dt_ in range(DM)]

            xc = [acts.tile([128, L], BF16, name=f"xc{m}", tag=f"xc{m}")
                  for m in range(EM)]
            xl = [acts.tile([128, L], BF16, name=f"xl{m}", tag=f"xl{m}")
                  for m in range(EM)]
            g = [acts.tile([128, L], BF16, name=f"g{m}", tag=f"g{m}")
                 for m in range(EM)]
            gp = [acts.tile([128, L], BF16, name=f"gp{m}", tag=f"gp{m}")
                  for m in range(EM)]
            osb = [acts.tile([128, L], F32, name=f"o{dt_}", tag=f"o{dt_}")
                   for dt_ in range(DM)]

            # decay broadcast tiles for the scan: aexp[m][p, :] = expA[m*128+p]
            # built as (in0 * 0) + expA ; in0 only supplies the shape
            aexp = []
            for m in range(EM):
                t = const.tile([128, CH], BF16, name=f"aexp{m}", tag=f"ae{m}")
                nc.vector.tensor_scalar(
                    out=t, in0=_bcast(v_scr, CH), scalar1=0.0,
                    scalar2=pts[m][:, PT_EXPA:PT_EXPA + 1],
                    op0=mybir.AluOpType.mult, op1=mybir.AluOpType.add)
                aexp.append(t)

            # ---- per channel-tile pipeline ----
            for m in range(EM):
                # mm1: pre1[e, l] = sum_d w1t[d, e] * xt[d, l]  (fp32r)
                for lc in range(LC):
                    ps1 = psA.tile([128, CH], F32, name="ps1", tag="ps1")
                    for k in range(DM):
                        nc.tensor.matmul(
                            out=ps1,
                            lhsT=w1s[k][:, m * 128:(m + 1) * 128],
                            rhs=xts[k][:, lc * CH:(lc + 1) * CH],
                            start=(k == 0), stop=(k == DM - 1))
                    # silu1 (+b1) -> xc bf16
                    nc.scalar.activation(
                        out=xc[m][:, lc * CH:(lc + 1) * CH], in_=ps1,
                        func=mybir.ActivationFunctionType.Silu,
                        bias=pts[m][:, PT_B1:PT_B1 + 1], scale=1.0)

                if m == 0:
                    # md touch: PE observes the diag/w2 DMA sem here, after mm1
                    nc.tensor.matmul(out=ps_scr[:, 4:8], lhsT=md_t[:, 0:128],
                                     rhs=md_t[:, 0:4], start=True, stop=True)
                # depthwise conv: taps outer (weight reuse), 4 banks live
                ps2 = [psB.tile([128, CH], F32, name=f"ps2_{m}_{lc}", tag="ps2")
                       for lc in range(LC)]
                for j, dlt in enumerate(TAPS):
                    tap = dlt + 2
                    lo, hi = max(0, -dlt), L - max(0, dlt)
                    for lc in range(LC):
                        c0, c1 = lc * CH, (lc + 1) * CH
                        a, b_ = max(c0, lo), min(c1, hi)
                        if a >= b_:
                            continue
                        nc.tensor.matmul(
                            out=ps2[lc][:, a - c0:b_ - c0],
                            lhsT=diag[m][tap],
                            rhs=xc[m][:, a + dlt:b_ + dlt],
                            start=(j == 0), stop=(j == len(TAPS) - 1),
                            skip_group_check=True)
                for lc in range(LC):
                    # silu2 (+conv/bn bias) -> xl bf16
                    nc.scalar.activation(
                        out=xl[m][:, lc * CH:(lc + 1) * CH], in_=ps2[lc],
                        func=mybir.ActivationFunctionType.Silu,
                        bias=pts[m][:, PT_CBIAS:PT_CBIAS + 1], scale=1.0)

            # chunked scan with carry: g[e,t] = expA[e]*g[e,t-1] + xl[e,t]
            # then fold both output paths into one tensor (on GpSimd):
            #   gp = (CB/Dv)*g + xl  so  out = w2dv @ gp
            # chunk-major order so mm2(lc) unlocks as early as possible
            for lc in range(LC):
                c0, c1 = lc * CH, (lc + 1) * CH
                for m in range(EM):
                    nc.vector.tensor_tensor_scan(
                        out=g[m][:, c0:c1], data0=aexp[m],
                        data1=xl[m][:, c0:c1],
                        initial=(0.0 if lc == 0 else g[m][:, c0 - 1:c0]),
                        op0=mybir.AluOpType.mult, op1=mybir.AluOpType.add)
                    nc.vector.scalar_tensor_tensor(
                        out=gp[m][:, c0:c1], in0=g[m][:, c0:c1],
                        scalar=pts[m][:, PT_CBDV:PT_CBDV + 1],
                        in1=xl[m][:, c0:c1],
                        op0=mybir.AluOpType.mult, op1=mybir.AluOpType.add)

            # ---- mm2 (bf16, single path): outT[d,l] = sum_e w2dv[e,d] gp[e,l] + b2
            for lc in range(LC):
                for dt_ in range(DM):
                    ps3 = psC.tile([128, CH], F32)
                    for ec in range(EM):
                        nc.tensor.matmul(
                            out=ps3,
                            lhsT=w2dvs[ec][:, dt_ * 128:(dt_ + 1) * 128],
                            rhs=gp[ec][:, lc * CH:(lc + 1) * CH],
                            start=(ec == 0), stop=(ec == EM - 1))
                    nc.scalar.activation(
                        out=osb[dt_][:, lc * CH:(lc + 1) * CH], in_=ps3,
                        func=mybir.ActivationFunctionType.Identity,
                        bias=b2s[dt_], scale=1.0)
                    nc.sync.dma_start(
                        out=outT[dt_ * 128:(dt_ + 1) * 128,
                                 lc * CH:(lc + 1) * CH],
                        in_=osb[dt_][:, lc * CH:(lc + 1) * CH])
    if wsplit:
        _split_waits(nc)
    return nc


_WSPLIT_SKIP = ("InstAllEngineBarrier", "InstNoOp",
                "InstEventSemaphore", "InstUnconditionalBranch")


def _split_waits(nc, max_waits=1):
    """Walrus codegen allows a single sync-wait command per TPB instruction.

    Move all-but-one waits of any over-limit instruction onto preceding
    NoOps (one wait each) on the same engine; same-engine program order
    makes this sound.
    """
    n_split = 0
    for f in nc.m.functions:
        for bb in f.blocks:
            out = []
            for inst in bb.instructions:
                si = inst.sync_info
                waits = list(si.on_wait) if si and si.on_wait else []
                if (len(waits) > max_waits
                        and inst.__class__.__name__ not in _WSPLIT_SKIP):
                    spill, keep = waits[:-max_waits], waits[-max_waits:]
                    for i, w in enumerate(spill):
                        out.append(mybir.InstNoOp(
                            name=f"{inst.name}_ws{i}",
                            engine=inst.engine,
                            sync_info=mybir.SyncInfo(on_wait=[w],
                                                     on_update=[]),
                        ))
                        n_split += 1
                    si.on_wait = keep
                out.append(inst)
            if n_split:
                bb.instructions = out
    return nc


def _to_bf16(a):
    import ml_dtypes
    return a.astype(ml_dtypes.bfloat16)


def host_params(w1, b1, wd, bd, gamma, beta, rmean, rvar, A, Bm, Cm, Dv, w2, b2):
    s = (gamma / np.sqrt(rvar + BN_EPS)).astype(np.float32)
    cw = (wd[:, 0, :] * s[:, None]).astype(np.float32)            # [E, 5]
    cbias = (bd * s + beta - rmean * s).astype(np.float32)        # [E]
    expA = np.exp(A).astype(np.float32)                           # [E]
    CB = (Bm * Cm).sum(1).astype(np.float32)                      # [E]
    w1t = np.asarray(w1, np.float32).T                            # [D, E]
    w2t = np.asarray(w2, np.float32).T                            # [E, D]

    mw = np.zeros((128, MW_COLS), np.float32)
    for k in range(DM):
        mw[:, k * 512:(k + 1) * 512] = w1t[k * 128:(k + 1) * 128, :]

    dv = np.asarray(Dv, np.float32).copy()
    tiny = np.abs(dv) < 1e-6
    dv[tiny] = np.where(dv[tiny] < 0, -1e-6, 1e-6)
    cbdv = CB / dv

    mdm = np.zeros((128, MD_COLS), np.float32)
    for m in range(EM):
        for j in range(5):
            c0 = (m * 5 + j) * 128
            np.fill_diagonal(mdm[:, c0:c0 + 128],
                             cw[m * 128:(m + 1) * 128, j])
    for ec in range(EM):
        blk = w2t[ec * 128:(ec + 1) * 128, :]
        mdm[:, W2DV0 + ec * 256:W2DV0 + (ec + 1) * 256] = \
            blk * dv[ec * 128:(ec + 1) * 128, None]
    mdm = _to_bf16(mdm)

    mpm = np.zeros((128, MP_COLS), np.float32)
    for m in range(EM):
        sl = slice(m * 128, (m + 1) * 128)
        mpm[:, m * PT_NCOL + 0:m * PT_NCOL + 5] = cw[sl]
        mpm[:, m * PT_NCOL + PT_CBIAS] = cbias[sl]
        mpm[:, m * PT_NCOL + PT_B1] = np.asarray(b1, np.float32)[sl]
        mpm[:, m * PT_NCOL + PT_EXPA] = expA[sl]
        mpm[:, m * PT_NCOL + PT_CBDV] = cbdv[sl]
    for dt_ in range(DM):
        mpm[:, EM * PT_NCOL + dt_] = \
            np.asarray(b2, np.float32)[dt_ * 128:(dt_ + 1) * 128]

    return dict(mw=mw, md=mdm, mp=mpm)


_CACHED_NC = None


def kernel(x, w1, b1, wd, bd, gamma, beta, rmean, rvar, A, Bm, Cm, Dv, w2, b2,
           **run_kwargs):
    from concourse.bass_utils import run_bass_kernel_spmd
    global _CACHED_NC
    if _CACHED_NC is None:
        _CACHED_NC = build_nc()
    nc = _CACHED_NC

    params = host_params(w1, b1, wd, bd, gamma, beta, rmean, rvar,
                         A, Bm, Cm, Dv, w2, b2)
    x = np.asarray(x, dtype=np.float32)
    in_maps = []
    for i in range(NCORES):
        m = dict(params)
        m["xt"] = np.ascontiguousarray(x[i].T)    # [D, L]
        in_maps.append(m)

    res = run_bass_kernel_spmd(nc, in_maps, core_ids=list(range(NCORES)),
                               **run_kwargs)
    out = np.stack([np.asarray(r["outT"]).T for r in res.results])  # [B, L, D]
    if run_kwargs:
        kernel.last_result = res
    return out


# revision 21
# speedup vs baseline: 1.0017x; 1.0017x over previous
"""MobileMamba block kernel for 8x Trainium2 NeuronCores.

Math restructure of the reference:
  xc   = silu(x @ w1.T + b1)                          # [L, E]
  c    = depthwise_conv5(xc) (+bd, BN affine folded)  # [E, L]
  xl   = silu(c)                                      # BN folded into taps/bias
  SSM with constant B/C collapses to a scalar first-order recurrence:
    g[e,t] = expA[e]*g[e,t-1] + xl[e,t]
    ys[e,t] = CB[e]*g[e,t] + Dv[e]*xl[e,t],  CB = sum_s Bm*Cm
  out  = ys @ w2.T + b2   (CB/Dv folded into two pre-scaled w2.T copies)

Sharding: data-parallel over batch (B=8 -> 8 cores). Each core computes one
sample entirely in [channel, time] layout; the host pre-transposes x shards
and post-transposes outputs.

Device pipeline per core (one batch sample):
  mm1 on TensorE (fp32r), silu on ScalarE, depthwise conv as 5 PSUM-
  accumulated diagonal matmuls (bf16) on TensorE, the SSM recurrence as
  chunked tensor_tensor_scan on VectorE (carry chained through the previous
  chunk's last column), mm2 (two-path: g and xl) on TensorE in bf16.

All constants arrive via 3 single-DMA mega tensors; tiny per-engine "touch"
ops observe those DMA semaphores early so every real instruction carries at
most one sync wait (walrus allows a single wait command per instruction;
_split_waits spills any excess onto same-engine NoOps).
"""

import sys

for _p in ('/opt/trn_rl_repo',):
    if _p not in sys.path:
        sys.path.append(_p)

import numpy as np

import concourse.bass as bass
import concourse.tile as tile
from concourse import mybir

D = 256      # model dim
E = 512      # expanded dim
L = 2048     # sequence length
B = 8        # batch
NCORES = 8
BN_EPS = 1e-5

F32 = mybir.dt.float32
F32R = mybir.dt.float32r
BF16 = mybir.dt.bfloat16

EM = E // 128   # 4 channel tiles
DM = D // 128   # 2 model-dim tiles

# param-table columns (per channel): taps 0..4, conv/bn bias, b1, expA, CB/Dv
PT_CBIAS = 5
PT_B1 = 6
PT_EXPA = 7
PT_CBDV = 8
PT_NCOL = 9

MD1_COLS = DM * 512                    # w1t chunks (bf16)
MD_COLS = EM * 5 * 128 + EM * 256      # diag blocks + w2dv (bf16)
MP_COLS = EM * PT_NCOL + DM
W2DV0 = EM * 5 * 128


def _bcast(col_ap, n):
    """Broadcast a [128,1] per-partition column AP along the free dim."""
    return bass.AP(tensor=col_ap.tensor, offset=col_ap.offset,
                   ap=[col_ap.ap[0], [0, n]])


def build_nc(L=L, wsplit=True):
    nc = bass.Bass()
    xt = nc.declare_dram_parameter("xt", [D, L], BF16, isOutput=False)
    md1 = nc.declare_dram_parameter("md1", [128, MD1_COLS], BF16, isOutput=False)
    md = nc.declare_dram_parameter("md", [128, MD_COLS], BF16, isOutput=False)
    mp = nc.declare_dram_parameter("mp", [128, MP_COLS], F32, isOutput=False)
    outT = nc.declare_dram_parameter("outT", [D, L], F32, isOutput=True)

    CH = min(512, L)
    LC = L // CH
    TAPS = (0, -1, 1, -2, 2)   # center first: start=True covers full range

    with tile.TileContext(nc) as tc:
        with (
            tc.tile_pool(name="const", bufs=1) as const,
            tc.tile_pool(name="acts", bufs=1) as acts,
            tc.tile_pool(name="psA", bufs=2, space="PSUM") as psA,
            tc.tile_pool(name="psB", bufs=4, space="PSUM") as psB,
            tc.tile_pool(name="psC", bufs=2, space="PSUM") as psC,
        ):
            # ---- constants: one DMA each ----
            mp_t = const.tile([128, MP_COLS], F32)
            nc.sync.dma_start(out=mp_t, in_=mp[:, :])
            mw_t = const.tile([128, MD1_COLS], BF16)
            nc.sync.dma_start(out=mw_t, in_=md1[:, :])

            # ---- x load, chunked, lc-major so mm1 can start early ----
            xts = [acts.tile([128, L], BF16, name=f"xts{k}", tag=f"xt{k}")
                   for k in range(DM)]
            md_t = const.tile([128, MD_COLS], BF16)
            for lc in range(LC):
                for k in range(DM):
                    nc.sync.dma_start(
                        out=xts[k][:, lc * CH:(lc + 1) * CH],
                        in_=xt[k * 128:(k + 1) * 128, lc * CH:(lc + 1) * CH])
                if lc == 0:
                    nc.sync.dma_start(out=md_t, in_=md[:, :])

            # ---- per-engine touches (observe const DMA sems early) ----
            ps_scr = psA.tile([128, 8], F32, name="ps_scr", tag="ps1")
            nc.tensor.matmul(out=ps_scr[:, 0:4], lhsT=mw_t[:, 0:128],
                             rhs=mw_t[:, 0:4], start=True, stop=True)
            v_scr = const.tile([128, 1], F32)
            nc.vector.tensor_copy(out=v_scr, in_=mp_t[:, 0:1])
            a_scr = const.tile([128, 1], F32)
            nc.scalar.copy(out=a_scr, in_=mp_t[:, 0:1])

            # ---- constant slices ----
            w1s = [mw_t[:, k * 512:(k + 1) * 512] for k in range(DM)]
            diag = [[md_t[:, (m * 5 + j) * 128:(m * 5 + j + 1) * 128]
                     for j in range(5)] for m in range(EM)]
            w2dvs = [md_t[:, W2DV0 + ec * 256:W2DV0 + (ec + 1) * 256]
                     for ec in range(EM)]
            pts = [mp_t[:, m * PT_NCOL:(m + 1) * PT_NCOL] for m in range(EM)]
            b2s = [mp_t[:, EM * PT_NCOL + dt_:EM * PT_NCOL + dt_ + 1]
                   for dt_ in range(DM)]

            xc = [acts.tile([128, L], BF16, name=f"xc{m}", tag=f"xc{m}")
                  for m in range(EM)]
            xl = [acts.tile([128, L], BF16, name=f"xl{m}", tag=f"xl{m}")
                  for m in range(EM)]
            g = [acts.tile([128, L], BF16, name=f"g{m}", tag=f"g{m}")
                 for m in range(EM)]
            gp = [acts.tile([128, L], BF16, name=f"gp{m}", tag=f"gp{m}")
                  for m in range(EM)]
            osb = [acts.tile([128, L], F32, name=f"o{dt_}", tag=f"o{dt_}")
                   for dt_ in range(DM)]

            # decay broadcast tiles for the scan: aexp[m][p, :] = expA[m*128+p]
            # built as (in0 * 0) + expA ; in0 only supplies the shape
            aexp = []
            for m in range(EM):
                t = const.tile([128, CH], BF16, name=f"aexp{m}", tag=f"ae{m}")
                nc.vector.tensor_scalar(
                    out=t, in0=_bcast(v_scr, CH), scalar1=0.0,
                    scalar2=pts[m][:, PT_EXPA:PT_EXPA + 1],
                    op0=mybir.AluOpType.mult, op1=mybir.AluOpType.add)
                aexp.append(t)

            # ---- per channel-tile pipeline ----
            for m in range(EM):
                # mm1: pre1[e, l] = sum_d w1t[d, e] * xt[d, l]  (fp32r)
                for lc in range(LC):
                    ps1 = psA.tile([128, CH], F32, name="ps1", tag="ps1")
                    for k in range(DM):
                        nc.tensor.matmul(
                            out=ps1,
                            lhsT=w1s[k][:, m * 128:(m + 1) * 128],
                            rhs=xts[k][:, lc * CH:(lc + 1) * CH],
                            start=(k == 0), stop=(k == DM - 1))
                    # silu1 (+b1) -> xc bf16
                    nc.scalar.activation(
                        out=xc[m][:, lc * CH:(lc + 1) * CH], in_=ps1,
                        func=mybir.ActivationFunctionType.Silu,
                        bias=pts[m][:, PT_B1:PT_B1 + 1], scale=1.0)

                if m == 0:
                    # md touch: PE observes the diag/w2 DMA sem here, after mm1
                    nc.tensor.matmul(out=ps_scr[:, 4:8], lhsT=md_t[:, 0:128],
                                     rhs=md_t[:, 0:4], start=True, stop=True)
                # depthwise conv: taps outer (weight reuse), 4 banks live
                ps2 = [psB.tile([128, CH], F32, name=f"ps2_{m}_{lc}", tag="ps2")
                       for lc in range(LC)]
                for j, dlt in enumerate(TAPS):
                    tap = dlt + 2
                    lo, hi = max(0, -dlt), L - max(0, dlt)
                    for lc in range(LC):
                        c0, c1 = lc * CH, (lc + 1) * CH
                        a, b_ = max(c0, lo), min(c1, hi)
                        if a >= b_:
                            continue
                        nc.tensor.matmul(
                            out=ps2[lc][:, a - c0:b_ - c0],
                            lhsT=diag[m][tap],
                            rhs=xc[m][:, a + dlt:b_ + dlt],
                            start=(j == 0), stop=(j == len(TAPS) - 1),
                            skip_group_check=True)
                for lc in range(LC):
                    # silu2 (+conv/bn bias) -> xl bf16
                    nc.scalar.activation(
                        out=xl[m][:, lc * CH:(lc + 1) * CH], in_=ps2[lc],
                        func=mybir.ActivationFunctionType.Silu,
                        bias=pts[m][:, PT_CBIAS:PT_CBIAS + 1], scale=1.0)

            # chunked scan with carry: g[e,t] = expA[e]*g[e,t-1] + xl[e,t]
            # then fold both output paths into one tensor (on GpSimd):
            #   gp = (CB/Dv)*g + xl  so  out = w2dv @ gp
            # chunk-major order so mm2(lc) unlocks as early as possible
            for lc in range(LC):
                c0, c1 = lc * CH, (lc + 1) * CH
                for m in range(EM):
                    nc.vector.tensor_tensor_scan(
                        out=g[m][:, c0:c1], data0=aexp[m],
                        data1=xl[m][:, c0:c1],
                        initial=(0.0 if lc == 0 else g[m][:, c0 - 1:c0]),
                        op0=mybir.AluOpType.mult, op1=mybir.AluOpType.add)
                    nc.vector.scalar_tensor_tensor(
                        out=gp[m][:, c0:c1], in0=g[m][:, c0:c1],
                        scalar=pts[m][:, PT_CBDV:PT_CBDV + 1],
                        in1=xl[m][:, c0:c1],
                        op0=mybir.AluOpType.mult, op1=mybir.AluOpType.add)

            # ---- mm2 (bf16, single path): outT[d,l] = sum_e w2dv[e,d] gp[e,l] + b2
            for lc in range(LC):
                for dt_ in range(DM):
                    ps3 = psC.tile([128, CH], F32)
                    for ec in range(EM):
                        nc.tensor.matmul(
                            out=ps3,
                            lhsT=w2dvs[ec][:, dt_ * 128:(dt_ + 1) * 128],
                            rhs=gp[ec][:, lc * CH:(lc + 1) * CH],
                            start=(ec == 0), stop=(ec == EM - 1))
                    nc.scalar.activation(
                        out=osb[dt_][:, lc * CH:(lc + 1) * CH], in_=ps3,
                        func=mybir.ActivationFunctionType.Identity,
                        bias=b2s[dt_], scale=1.0)
                    nc.sync.dma_start(
                        out=outT[dt_ * 128:(dt_ + 1) * 128,
                                 lc * CH:(lc + 1) * CH],
                        in_=osb[dt_][:, lc * CH:(lc + 1) * CH])
    if wsplit:
        _split_waits(nc)
    return nc


_WSPLIT_SKIP = ("InstAllEngineBarrier", "InstNoOp",
                "InstEventSemaphore", "InstUnconditionalBranch")


def _split_waits(nc, max_waits=1):
    """Walrus codegen allows a single sync-wait command per TPB instruction.

    Move all-but-one waits of any over-limit instruction onto preceding
    NoOps (one wait each) on the same engine; same-engine program order
    makes this sound.
    """
    n_split = 0
    for f in nc.m.functions:
        for bb in f.blocks:
            out = []
            for inst in bb.instructions:
                si = inst.sync_info
                waits = list(si.on_wait) if si and si.on_wait else []
                if (len(waits) > max_waits
                        and inst.__class__.__name__ not in _WSPLIT_SKIP):
                    spill, keep = waits[:-max_waits], waits[-max_waits:]
                    for i, w in enumerate(spill):
                        out.append(mybir.InstNoOp(
                            name=f"{inst.name}_ws{i}",
                            engine=inst.engine,
                            sync_info=mybir.SyncInfo(on_wait=[w],
                                                     on_update=[]),
                        ))
                        n_split += 1
                    si.on_wait = keep
                out.append(inst)
            if n_split:
                bb.instructions = out
    return nc


def _to_bf16(a):
    import ml_dtypes
    return a.astype(ml_dtypes.bfloat16)


def host_params(w1, b1, wd, bd, gamma, beta, rmean, rvar, A, Bm, Cm, Dv, w2, b2):
    s = (gamma / np.sqrt(rvar + BN_EPS)).astype(np.float32)
    cw = (wd[:, 0, :] * s[:, None]).astype(np.float32)            # [E, 5]
    cbias = (bd * s + beta - rmean * s).astype(np.float32)        # [E]
    expA = np.exp(A).astype(np.float32)                           # [E]
    CB = (Bm * Cm).sum(1).astype(np.float32)                      # [E]
    w1t = np.asarray(w1, np.float32).T                            # [D, E]
    w2t = np.asarray(w2, np.float32).T                            # [E, D]

    md1 = np.zeros((128, MD1_COLS), np.float32)
    for k in range(DM):
        md1[:, k * 512:(k + 1) * 512] = w1t[k * 128:(k + 1) * 128, :]
    md1 = _to_bf16(md1)

    dv = np.asarray(Dv, np.float32).copy()
    tiny = np.abs(dv) < 1e-6
    dv[tiny] = np.where(dv[tiny] < 0, -1e-6, 1e-6)
    cbdv = CB / dv

    mdm = np.zeros((128, MD_COLS), np.float32)
    for m in range(EM):
        for j in range(5):
            c0 = (m * 5 + j) * 128
            np.fill_diagonal(mdm[:, c0:c0 + 128],
                             cw[m * 128:(m + 1) * 128, j])
    for ec in range(EM):
        blk = w2t[ec * 128:(ec + 1) * 128, :]
        mdm[:, W2DV0 + ec * 256:W2DV0 + (ec + 1) * 256] = \
            blk * dv[ec * 128:(ec + 1) * 128, None]
    mdm = _to_bf16(mdm)

    mpm = np.zeros((128, MP_COLS), np.float32)
    for m in range(EM):
        sl = slice(m * 128, (m + 1) * 128)
        mpm[:, m * PT_NCOL + 0:m * PT_NCOL + 5] = cw[sl]
        mpm[:, m * PT_NCOL + PT_CBIAS] = cbias[sl]
        mpm[:, m * PT_NCOL + PT_B1] = np.asarray(b1, np.float32)[sl]
        mpm[:, m * PT_NCOL + PT_EXPA] = expA[sl]
        mpm[:, m * PT_NCOL + PT_CBDV] = cbdv[sl]
    for dt_ in range(DM):
        mpm[:, EM * PT_NCOL + dt_] = \
            np.asarray(b2, np.float32)[dt_ * 128:(dt_ + 1) * 128]

    return dict(md1=md1, md=mdm, mp=mpm)


_CACHED_NC = None


def kernel(x, w1, b1, wd, bd, gamma, beta, rmean, rvar, A, Bm, Cm, Dv, w2, b2,
           **run_kwargs):
    from concourse.bass_utils import run_bass_kernel_spmd
    global _CACHED_NC
    if _CACHED_NC is None:
        _CACHED_NC = build_nc()
    nc = _CACHED_NC

    params = host_params(w1, b1, wd, bd, gamma, beta, rmean, rvar,
                         A, Bm, Cm, Dv, w2, b2)
    x = np.asarray(x, dtype=np.float32)
    in_maps = []
    for i in range(NCORES):
        m = dict(params)
        m["xt"] = _to_bf16(np.ascontiguousarray(x[i].T))  # [D, L] bf16
        in_maps.append(m)

    res = run_bass_kernel_spmd(nc, in_maps, core_ids=list(range(NCORES)),
                               **run_kwargs)
    out = np.stack([np.asarray(r["outT"]).T for r in res.results])  # [B, L, D]
    if run_kwargs:
        kernel.last_result = res
    return out


# revision 23
# speedup vs baseline: 1.1203x; 1.1184x over previous
"""MobileMamba block kernel for 8x Trainium2 NeuronCores.

Math restructure of the reference:
  xc   = silu(x @ w1.T + b1)                          # [L, E]
  c    = depthwise_conv5(xc) (+bd, BN affine folded)  # [E, L]
  xl   = silu(c)                                      # BN folded into taps/bias
  SSM with constant B/C collapses to a scalar first-order recurrence:
    g[e,t] = expA[e]*g[e,t-1] + xl[e,t]
    ys[e,t] = CB[e]*g[e,t] + Dv[e]*xl[e,t],  CB = sum_s Bm*Cm
  out  = ys @ w2.T + b2   (CB/Dv folded into two pre-scaled w2.T copies)

Sharding: data-parallel over batch (B=8 -> 8 cores). Each core computes one
sample entirely in [channel, time] layout; the host pre-transposes x shards
and post-transposes outputs.

Device pipeline per core (one batch sample):
  mm1 on TensorE (fp32r), silu on ScalarE, depthwise conv as 5 PSUM-
  accumulated diagonal matmuls (bf16) on TensorE, the SSM recurrence as
  chunked tensor_tensor_scan on VectorE (carry chained through the previous
  chunk's last column), mm2 (two-path: g and xl) on TensorE in bf16.

All constants arrive via 3 single-DMA mega tensors; tiny per-engine "touch"
ops observe those DMA semaphores early so every real instruction carries at
most one sync wait (walrus allows a single wait command per instruction;
_split_waits spills any excess onto same-engine NoOps).
"""

import sys

for _p in ('/opt/trn_rl_repo',):
    if _p not in sys.path:
        sys.path.append(_p)

import numpy as np

import concourse.bass as bass
import concourse.tile as tile
from concourse import mybir

D = 256      # model dim
E = 512      # expanded dim
L = 2048     # sequence length
B = 8        # batch
NCORES = 8
BN_EPS = 1e-5

F32 = mybir.dt.float32
F32R = mybir.dt.float32r
BF16 = mybir.dt.bfloat16

EM = E // 128   # 4 channel tiles
DM = D // 128   # 2 model-dim tiles

# param-table columns (per channel): taps 0..4, conv/bn bias, b1, expA, CB/Dv
PT_CBIAS = 5
PT_B1 = 6
PT_EXPA = 7
PT_CBDV = 8
PT_NCOL = 9

MD1_COLS = DM * 512                    # w1t chunks (bf16)
MD_COLS = EM * 5 * 128 + EM * 256      # diag blocks + w2dv (bf16)
MP_COLS = EM * PT_NCOL + DM
W2DV0 = EM * 5 * 128


def _bcast(col_ap, n):
    """Broadcast a [128,1] per-partition column AP along the free dim."""
    return bass.AP(tensor=col_ap.tensor, offset=col_ap.offset,
                   ap=[col_ap.ap[0], [0, n]])


def build_nc(L=L, wsplit=True):
    nc = bass.Bass()
    xt = nc.declare_dram_parameter("xt", [D, L], BF16, isOutput=False)
    md1 = nc.declare_dram_parameter("md1", [128, MD1_COLS], BF16, isOutput=False)
    md = nc.declare_dram_parameter("md", [128, MD_COLS], BF16, isOutput=False)
    mp = nc.declare_dram_parameter("mp", [128, MP_COLS], F32, isOutput=False)
    outT = nc.declare_dram_parameter("outT", [D, L], F32, isOutput=True)

    CH = min(512, L)
    LC = L // CH
    TAPS = (0, -1, 1, -2, 2)   # center first: start=True covers full range

    with tile.TileContext(nc) as tc:
        with (
            tc.tile_pool(name="const", bufs=1) as const,
            tc.tile_pool(name="acts", bufs=1) as acts,
            tc.tile_pool(name="psA", bufs=3, space="PSUM") as psA,
            tc.tile_pool(name="psB", bufs=3, space="PSUM") as psB,
            tc.tile_pool(name="psC", bufs=2, space="PSUM") as psC,
        ):
            # ---- constants: one DMA each ----
            mp_t = const.tile([128, MP_COLS], F32)
            nc.sync.dma_start(out=mp_t, in_=mp[:, :])
            mw_t = const.tile([128, MD1_COLS], BF16)
            nc.sync.dma_start(out=mw_t, in_=md1[:, :])

            # ---- x load, chunked, lc-major so mm1 can start early ----
            xts = [acts.tile([128, L], BF16, name=f"xts{k}", tag=f"xt{k}")
                   for k in range(DM)]
            md_t = const.tile([128, MD_COLS], BF16)
            for lc in range(LC):
                for k in range(DM):
                    nc.sync.dma_start(
                        out=xts[k][:, lc * CH:(lc + 1) * CH],
                        in_=xt[k * 128:(k + 1) * 128, lc * CH:(lc + 1) * CH])
                if lc == 0:
                    nc.sync.dma_start(out=md_t, in_=md[:, :])

            # ---- per-engine touches (observe const DMA sems early) ----
            ps_scr = psA.tile([128, 8], F32, name="ps_scr", tag="ps1")
            nc.tensor.matmul(out=ps_scr[:, 0:4], lhsT=mw_t[:, 0:128],
                             rhs=mw_t[:, 0:4], start=True, stop=True)
            v_scr = const.tile([128, 1], F32)
            nc.vector.tensor_copy(out=v_scr, in_=mp_t[:, 0:1])
            a_scr = const.tile([128, 1], F32)
            nc.scalar.copy(out=a_scr, in_=mp_t[:, 0:1])

            # ---- constant slices ----
            w1s = [mw_t[:, k * 512:(k + 1) * 512] for k in range(DM)]
            diag = [[md_t[:, (m * 5 + j) * 128:(m * 5 + j + 1) * 128]
                     for j in range(5)] for m in range(EM)]
            w2dvs = [md_t[:, W2DV0 + ec * 256:W2DV0 + (ec + 1) * 256]
                     for ec in range(EM)]
            pts = [mp_t[:, m * PT_NCOL:(m + 1) * PT_NCOL] for m in range(EM)]
            b2s = [mp_t[:, EM * PT_NCOL + dt_:EM * PT_NCOL + dt_ + 1]
                   for dt_ in range(DM)]

            xc = [acts.tile([128, L], BF16, name=f"xc{m}", tag=f"xc{m}")
                  for m in range(EM)]
            xl = [acts.tile([128, L], BF16, name=f"xl{m}", tag=f"xl{m}")
                  for m in range(EM)]
            g = [acts.tile([128, L], BF16, name=f"g{m}", tag=f"g{m}")
                 for m in range(EM)]
            gp = [acts.tile([128, L], BF16, name=f"gp{m}", tag=f"gp{m}")
                  for m in range(EM)]
            osb = [acts.tile([128, L], F32, name=f"o{dt_}", tag=f"o{dt_}")
                   for dt_ in range(DM)]

            # decay broadcast tiles for the scan: aexp[m][p, :] = expA[m*128+p]
            # built as (in0 * 0) + expA ; in0 only supplies the shape
            aexp = []
            for m in range(EM):
                t = const.tile([128, CH], BF16, name=f"aexp{m}", tag=f"ae{m}")
                nc.scalar.activation(
                    out=t, in_=_bcast(a_scr, CH),
                    func=mybir.ActivationFunctionType.Identity,
                    bias=pts[m][:, PT_EXPA:PT_EXPA + 1], scale=0.0)
                aexp.append(t)

            # ---- per channel-tile pipeline ----
            # conv chunk lc reads 2 halo columns of xc from chunk lc+1, so
            # the tail stages run one chunk behind mm1/silu1.
            def mm1_stage(m, lc):
                c0, c1 = lc * CH, (lc + 1) * CH
                ps1 = psA.tile([128, CH], F32, name="ps1", tag="ps1")
                for k in range(DM):
                    nc.tensor.matmul(
                        out=ps1,
                        lhsT=w1s[k][:, m * 128:(m + 1) * 128],
                        rhs=xts[k][:, c0:c1],
                        start=(k == 0), stop=(k == DM - 1))
                nc.scalar.activation(
                    out=xc[m][:, c0:c1], in_=ps1,
                    func=mybir.ActivationFunctionType.Silu,
                    bias=pts[m][:, PT_B1:PT_B1 + 1], scale=1.0)

            def tail_stage(m, lc):
                c0, c1 = lc * CH, (lc + 1) * CH
                ps2 = psB.tile([128, CH], F32, name="ps2", tag="ps2")
                for j, dlt in enumerate(TAPS):
                    tap = dlt + 2
                    lo, hi = max(0, -dlt), L - max(0, dlt)
                    a, b_ = max(c0, lo), min(c1, hi)
                    if a >= b_:
                        continue
                    nc.tensor.matmul(
                        out=ps2[:, a - c0:b_ - c0],
                        lhsT=diag[m][tap],
                        rhs=xc[m][:, a + dlt:b_ + dlt],
                        start=(j == 0), stop=(j == len(TAPS) - 1),
                        skip_group_check=True)
                nc.scalar.activation(
                    out=xl[m][:, c0:c1], in_=ps2,
                    func=mybir.ActivationFunctionType.Silu,
                    bias=pts[m][:, PT_CBIAS:PT_CBIAS + 1], scale=1.0)
                # scan chunk (carry-chained), then fold both output paths:
                #   gp = (CB/Dv)*g + xl  so  out = w2dv @ gp
                nc.vector.tensor_tensor_scan(
                    out=g[m][:, c0:c1], data0=aexp[m],
                    data1=xl[m][:, c0:c1],
                    initial=(0.0 if lc == 0 else g[m][:, c0 - 1:c0]),
                    op0=mybir.AluOpType.mult, op1=mybir.AluOpType.add)
                nc.vector.scalar_tensor_tensor(
                    out=gp[m][:, c0:c1], in0=g[m][:, c0:c1],
                    scalar=pts[m][:, PT_CBDV:PT_CBDV + 1],
                    in1=xl[m][:, c0:c1],
                    op0=mybir.AluOpType.mult, op1=mybir.AluOpType.add)

            for m in range(EM):
                if m == 0:
                    # md touch: PE observes the diag/w2 DMA sem here
                    nc.tensor.matmul(out=ps_scr[:, 4:8], lhsT=md_t[:, 0:128],
                                     rhs=md_t[:, 0:4], start=True, stop=True)
                mm1_stage(m, 0)
                for lc in range(1, LC):
                    mm1_stage(m, lc)
                    tail_stage(m, lc - 1)
                tail_stage(m, LC - 1)

            # ---- mm2 (bf16, single path): outT[d,l] = sum_e w2dv[e,d] gp[e,l] + b2
            for lc in range(LC):
                for dt_ in range(DM):
                    ps3 = psC.tile([128, CH], F32)
                    for ec in range(EM):
                        nc.tensor.matmul(
                            out=ps3,
                            lhsT=w2dvs[ec][:, dt_ * 128:(dt_ + 1) * 128],
                            rhs=gp[ec][:, lc * CH:(lc + 1) * CH],
                            start=(ec == 0), stop=(ec == EM - 1))
                    nc.scalar.activation(
                        out=osb[dt_][:, lc * CH:(lc + 1) * CH], in_=ps3,
                        func=mybir.ActivationFunctionType.Identity,
                        bias=b2s[dt_], scale=1.0)
                    nc.sync.dma_start(
                        out=outT[dt_ * 128:(dt_ + 1) * 128,
                                 lc * CH:(lc + 1) * CH],
                        in_=osb[dt_][:, lc * CH:(lc + 1) * CH])
    if wsplit:
        _split_waits(nc)
    return nc


_WSPLIT_SKIP = ("InstAllEngineBarrier", "InstNoOp",
                "InstEventSemaphore", "InstUnconditionalBranch")


def _split_waits(nc, max_waits=1):
    """Walrus codegen allows a single sync-wait command per TPB instruction.

    Move all-but-one waits of any over-limit instruction onto preceding
    NoOps (one wait each) on the same engine; same-engine program order
    makes this sound.
    """
    n_split = 0
    for f in nc.m.functions:
        for bb in f.blocks:
            out = []
            for inst in bb.instructions:
                si = inst.sync_info
                waits = list(si.on_wait) if si and si.on_wait else []
                if (len(waits) > max_waits
                        and inst.__class__.__name__ not in _WSPLIT_SKIP):
                    spill, keep = waits[:-max_waits], waits[-max_waits:]
                    for i, w in enumerate(spill):
                        out.append(mybir.InstNoOp(
                            name=f"{inst.name}_ws{i}",
                            engine=inst.engine,
                            sync_info=mybir.SyncInfo(on_wait=[w],
                                                     on_update=[]),
                        ))
                        n_split += 1
                    si.on_wait = keep
                out.append(inst)
            if n_split:
                bb.instructions = out
    return nc


def _to_bf16(a):
    import ml_dtypes
    return a.astype(ml_dtypes.bfloat16)


def host_params(w1, b1, wd, bd, gamma, beta, rmean, rvar, A, Bm, Cm, Dv, w2, b2):
    s = (gamma / np.sqrt(rvar + BN_EPS)).astype(np.float32)
    cw = (wd[:, 0, :] * s[:, None]).astype(np.float32)            # [E, 5]
    cbias = (bd * s + beta - rmean * s).astype(np.float32)        # [E]
    expA = np.exp(A).astype(np.float32)                           # [E]
    CB = (Bm * Cm).sum(1).astype(np.float32)                      # [E]
    w1t = np.asarray(w1, np.float32).T                            # [D, E]
    w2t = np.asarray(w2, np.float32).T                            # [E, D]

    md1 = np.zeros((128, MD1_COLS), np.float32)
    for k in range(DM):
        md1[:, k * 512:(k + 1) * 512] = w1t[k * 128:(k + 1) * 128, :]
    md1 = _to_bf16(md1)

    dv = np.asarray(Dv, np.float32).copy()
    tiny = np.abs(dv) < 1e-6
    dv[tiny] = np.where(dv[tiny] < 0, -1e-6, 1e-6)
    cbdv = CB / dv

    mdm = np.zeros((128, MD_COLS), np.float32)
    for m in range(EM):
        for j in range(5):
            c0 = (m * 5 + j) * 128
            np.fill_diagonal(mdm[:, c0:c0 + 128],
                             cw[m * 128:(m + 1) * 128, j])
    for ec in range(EM):
        blk = w2t[ec * 128:(ec + 1) * 128, :]
        mdm[:, W2DV0 + ec * 256:W2DV0 + (ec + 1) * 256] = \
            blk * dv[ec * 128:(ec + 1) * 128, None]
    mdm = _to_bf16(mdm)

    mpm = np.zeros((128, MP_COLS), np.float32)
    for m in range(EM):
        sl = slice(m * 128, (m + 1) * 128)
        mpm[:, m * PT_NCOL + 0:m * PT_NCOL + 5] = cw[sl]
        mpm[:, m * PT_NCOL + PT_CBIAS] = cbias[sl]
        mpm[:, m * PT_NCOL + PT_B1] = np.asarray(b1, np.float32)[sl]
        mpm[:, m * PT_NCOL + PT_EXPA] = expA[sl]
        mpm[:, m * PT_NCOL + PT_CBDV] = cbdv[sl]
    for dt_ in range(DM):
        mpm[:, EM * PT_NCOL + dt_] = \
            np.asarray(b2, np.float32)[dt_ * 128:(dt_ + 1) * 128]

    return dict(md1=md1, md=mdm, mp=mpm)


_CACHED_NC = None


def kernel(x, w1, b1, wd, bd, gamma, beta, rmean, rvar, A, Bm, Cm, Dv, w2, b2,
           **run_kwargs):
    from concourse.bass_utils import run_bass_kernel_spmd
    global _CACHED_NC
    if _CACHED_NC is None:
        _CACHED_NC = build_nc()
    nc = _CACHED_NC

    params = host_params(w1, b1, wd, bd, gamma, beta, rmean, rvar,
                         A, Bm, Cm, Dv, w2, b2)
    x = np.asarray(x, dtype=np.float32)
    in_maps = []
    for i in range(NCORES):
        m = dict(params)
        m["xt"] = _to_bf16(np.ascontiguousarray(x[i].T))  # [D, L] bf16
        in_maps.append(m)

    res = run_bass_kernel_spmd(nc, in_maps, core_ids=list(range(NCORES)),
                               **run_kwargs)
    out = np.stack([np.asarray(r["outT"]).T for r in res.results])  # [B, L, D]
    if run_kwargs:
        kernel.last_result = res
    return out
